# revision 3
# baseline (speedup 1.0000x reference)
"""Trainium2 Bass kernel for nn_BertAdapterAttentionMask (v3: fp8 DR + sparsity).

Math restructuring:
  * Query is a broadcast task embedding -> scores GEMM is rank-hd with a
    column-duplicated Mk so probs come out partition-duplicated for free.
  * ALL sigmoid gates saturate (s = SMAX = 400), so per-task output gates
    g2 are ~binary. Channels with g2 < 1e-3 are dropped exactly: per task
    the active H-channels are gathered into compact chunks via host-side
    gathers of fc2-output columns / Wv rows / Mk rows (g2 folded into the
    gathered weights). fc2/scores/V shrink from 8 chunks to 4-5 per task.
  * GEMM operands in fp8-e4m3 (TRN FP8_EXP4, max 240) with power-of-2
    per-tensor scales; matmuls in DoubleRow perf mode (2 fp8 weights per
    PE cell). Descales ride the activation `scale` operand.
  * Device ships the unnormalized attention numerator (sum_t e_t * v_t)
    and denominator (sum_t e_t); softmax division, head permutation,
    V-bias and residual-x add happen on the host.

Scheduling notes (from trace): ACT table switches cost 1.28us, so scores
are drained from PSUM with a table-free Copy and all exps run batched.
fc1/fc2 use 2-bank PSUM tiles with paired (2-wide) gelu/exp activations.
GPSIMD tensor ops are ~1.27us/[128,512] (3x DVE), so it only gets the
side branches of the task-sum tree plus the denominator.
"""

import os
import numpy as np
import ml_dtypes
from contextlib import ExitStack

import concourse.bass as bass
import concourse.tile as tile
from concourse import bacc, mybir
from concourse.bass_utils import run_bass_kernel_spmd

AF = mybir.ActivationFunctionType
BF16 = mybir.dt.bfloat16
F32 = mybir.dt.float32
FP8 = mybir.dt.float8e4
NPBF16 = ml_dtypes.bfloat16
NPFP8 = ml_dtypes.float8_e4m3
DR = mybir.MatmulPerfMode.DoubleRow

B, S, H, A, NH, HD = 8, 2048, 1024, 512, 16, 64
T = 6              # tasks = t + 1
P = 128
ST = 512           # s-tile (free-dim tile)
NST = S // ST      # 4
NHC = H // P       # 8
NAC = A // P       # 4
SMAX = 400.0
THR = 1e-3         # drop channels with g2 below this (exact to ~1e-4 abs)

USE_FP8 = os.environ.get("KBENCH_FP8", "1") == "1"

_CACHE = {}


def _build_nc(c_list, use_fp8, zero_bias):
    c_list = list(c_list)
    CSUM = sum(c_list)
    off = [0]
    for c in c_list:
        off.append(off[-1] + c)
    QD = FP8 if use_fp8 else BF16

    nc = bacc.Bacc("TRN2", target_bir_lowering=False, debug=False)

    d_xT = nc.dram_tensor("xT", [H, S], QD, kind="ExternalInput").ap()
    d_w1 = nc.dram_tensor("w1", [NAC, P, 2, A], QD, kind="ExternalInput").ap()
    d_b1 = nc.dram_tensor("b1", [P, NAC], F32, kind="ExternalInput").ap()
    d_w2 = nc.dram_tensor("w2", [P, NAC, CSUM * P], QD, kind="ExternalInput").ap()
    d_b2 = nc.dram_tensor("b2", [P, CSUM], F32, kind="ExternalInput").ap()
    d_mk = nc.dram_tensor("mk", [P, CSUM, P], QD, kind="ExternalInput").ap()
    d_wv = nc.dram_tensor("wv", [P, CSUM, H], QD, kind="ExternalInput").ap()
    d_ck = nc.dram_tensor("ck", [P, 1], F32, kind="ExternalInput").ap()
    d_scl = nc.dram_tensor("scl", [P, 3], F32, kind="ExternalInput").ap()
    d_num = nc.dram_tensor("num", [H, S], F32, kind="ExternalOutput").ap()
    d_den = nc.dram_tensor("den", [HD, S], F32, kind="ExternalOutput").ap()

    def mm_pairs(ps_ap, wt_slices, rhs_slices, tail, perf8):
        """Emit an accumulation group: DR pairs + optional single tail."""
        n = len(wt_slices)
        for i, (w_ap, r_ap) in enumerate(zip(wt_slices, rhs_slices)):
            if perf8:
                nc.tensor.matmul(ps_ap, w_ap, r_ap,
                                 start=(i == 0),
                                 stop=(tail is None and i == n - 1),
                                 perf_mode=DR)
            else:
                raise AssertionError
        if tail is not None:
            nc.tensor.matmul(ps_ap, tail[0], tail[1],
                             start=(n == 0), stop=True)

    with tile.TileContext(nc) as tc:
        with ExitStack() as ctx:
            wp = ctx.enter_context(tc.tile_pool(name="weights", bufs=1))
            xp = ctx.enter_context(tc.tile_pool(name="acts", bufs=2))
            psp = ctx.enter_context(
                tc.tile_pool(name="psum", bufs=2, space="PSUM")
            )

            # ---- resident weights (DMA order = first-use order) ----
            w1t = []
            xt0 = []
            for kp in range(NAC):
                t_ = wp.tile([P, 2, A], QD, tag=f"w1_{kp}")
                nc.sync.dma_start(t_[:], d_w1[kp])
                w1t.append(t_)
                t_ = xp.tile([P, 2, ST], QD, name=f"xt{kp}", tag=f"xt_{kp}", bufs=2)
                nc.sync.dma_start(t_[:, 0, :], d_xT[kp * 2 * P:kp * 2 * P + P, 0:ST])
                nc.sync.dma_start(t_[:, 1, :], d_xT[kp * 2 * P + P:(kp + 1) * 2 * P, 0:ST])
                xt0.append(t_)
            b1t = wp.tile([P, NAC], F32, tag="b1")
            nc.sync.dma_start(b1t[:], d_b1)
            sclt = wp.tile([P, 3], F32, tag="scl")
            nc.sync.dma_start(sclt[:], d_scl)
            ckt = wp.tile([P, 1], F32, tag="ck")
            nc.sync.dma_start(ckt[:], d_ck)
            w2t = wp.tile([P, NAC, CSUM * P], QD, tag="w2")
            for p in range(T):
                nc.sync.dma_start(
                    w2t[:, :, off[p] * P:off[p + 1] * P],
                    d_w2[:, :, off[p] * P:off[p + 1] * P],
                )
            b2t = wp.tile([P, CSUM], F32, tag="b2")
            nc.sync.dma_start(b2t[:], d_b2)
            mkt = wp.tile([P, CSUM, P], QD, tag="mk")
            nc.sync.dma_start(mkt[:], d_mk)
            wvt = wp.tile([P, CSUM, H], QD, tag="wv")
            for p in range(T):
                nc.sync.dma_start(
                    wvt[:, off[p]:off[p + 1], :], d_wv[:, off[p]:off[p + 1], :]
                )

            for st in range(NST):
                s0 = st * ST
                if st == 0:
                    xt = xt0
                else:
                    xt = []
                    for kp in range(NAC):
                        t_ = xp.tile([P, 2, ST], QD, name=f"xt{kp}",
                                     tag=f"xt_{kp}", bufs=2)
                        nc.sync.dma_start(
                            t_[:, 0, :],
                            d_xT[kp * 2 * P:kp * 2 * P + P, s0:s0 + ST])
                        nc.sync.dma_start(
                            t_[:, 1, :],
                            d_xT[kp * 2 * P + P:(kp + 1) * 2 * P, s0:s0 + ST])
                        xt.append(t_)

                # ---- fc1 -> h1 (gelu), 2-wide PSUM ----
                h1 = xp.tile([P, NAC, ST], QD, name="h1", tag="h1", bufs=2)
                for a2 in range(NAC // 2):
                    ps2 = psp.tile([P, 2, ST], F32, tag="ps_mm", bufs=2)
                    for half in range(2):
                        ac = a2 * 2 + half
                        for kp in range(NAC):
                            if use_fp8:
                                nc.tensor.matmul(
                                    ps2[:, half, :],
                                    w1t[kp][:, :, ac * P:(ac + 1) * P],
                                    xt[kp][:],
                                    start=(kp == 0), stop=(kp == NAC - 1),
                                    perf_mode=DR)
                            else:
                                for i in range(2):
                                    nc.tensor.matmul(
                                        ps2[:, half, :],
                                        w1t[kp][:, i, ac * P:(ac + 1) * P],
                                        xt[kp][:, i, :],
                                        start=(kp == 0 and i == 0),
                                        stop=(kp == NAC - 1 and i == 1))
                    if zero_bias:
                        nc.scalar.activation(
                            h1[:, a2 * 2:a2 * 2 + 2, :], ps2[:], AF.Gelu,
                            scale=sclt[:, 0:1])
                    else:
                        for half in range(2):
                            ac = a2 * 2 + half
                            nc.scalar.activation(
                                h1[:, ac, :], ps2[:, half, :], AF.Gelu,
                                bias=b1t[:, ac:ac + 1], scale=sclt[:, 0:1])

                # ---- fc2 per task (2-wide) + scores interleaved ----
                gst = xp.tile([P, CSUM, ST], QD, name="gst", tag="gst", bufs=1)
                ssc = xp.tile([P, T, ST], F32, name="ssc", tag="ssc", bufs=1)
                for pp in range(T // 2):   # task pairs for scores PSUM
                    ps_s2 = psp.tile([P, 2, ST], F32, tag="ps_s", bufs=1)
                    for half2 in range(2):
                        p = pp * 2 + half2
                        c = c_list[p]
                        o = off[p]
                        # fc2 blocks, paired
                        for c2 in range(0, c, 2):
                            wide = 2 if c2 + 1 < c else 1
                            ps2 = psp.tile([P, 2, ST], F32, tag="ps_mm", bufs=2)
                            for half in range(wide):
                                j = o + c2 + half
                                if use_fp8:
                                    nc.tensor.matmul(
                                        ps2[:, half, :],
                                        w2t[:, 0:2, j * P:(j + 1) * P],
                                        h1[:, 0:2, :], start=True, stop=False,
                                        perf_mode=DR)
                                    nc.tensor.matmul(
                                        ps2[:, half, :],
                                        w2t[:, 2:4, j * P:(j + 1) * P],
                                        h1[:, 2:4, :], start=False, stop=True,
                                        perf_mode=DR)
                                else:
                                    for a4 in range(NAC):
                                        nc.tensor.matmul(
                                            ps2[:, half, :],
                                            w2t[:, a4, j * P:(j + 1) * P],
                                            h1[:, a4, :],
                                            start=(a4 == 0),
                                            stop=(a4 == NAC - 1))
                            if zero_bias and wide == 2:
                                nc.scalar.activation(
                                    gst[:, o + c2:o + c2 + 2, :], ps2[:],
                                    AF.Gelu, scale=sclt[:, 1:2])
                            else:
                                for half in range(wide):
                                    j = o + c2 + half
                                    nc.scalar.activation(
                                        gst[:, j, :], ps2[:, half, :], AF.Gelu,
                                        bias=b2t[:, j:j + 1], scale=sclt[:, 1:2])
                        # scores for task p into ps_s2[:, half2, :]
                        if use_fp8:
                            for i in range(c // 2):
                                nc.tensor.matmul(
                                    ps_s2[:, half2, :],
                                    mkt[:, o + 2 * i:o + 2 * i + 2, :],
                                    gst[:, o + 2 * i:o + 2 * i + 2, :],
                                    start=(i == 0),
                                    stop=(c % 2 == 0 and i == c // 2 - 1),
                                    perf_mode=DR)
                            if c % 2:
                                nc.tensor.matmul(
                                    ps_s2[:, half2, :], mkt[:, o + c - 1, :],
                                    gst[:, o + c - 1, :],
                                    start=(c // 2 == 0), stop=True)
                        else:
                            for i in range(c):
                                nc.tensor.matmul(
                                    ps_s2[:, half2, :], mkt[:, o + i, :],
                                    gst[:, o + i, :],
                                    start=(i == 0), stop=(i == c - 1))
                    # drain scores PSUM with a table-free Copy (no Exp yet)
                    nc.scalar.activation(
                        ssc[:, pp * 2:pp * 2 + 2, :], ps_s2[:], AF.Copy)

                # ---- batched exps (one ACT table switch) ----
                e_t = xp.tile([P, T, ST], F32, name="e", tag="e", bufs=2)
                for pp in range(T // 2):
                    nc.scalar.activation(
                        e_t[:, pp * 2:pp * 2 + 2, :],
                        ssc[:, pp * 2:pp * 2 + 2, :],
                        AF.Exp, bias=ckt[:], scale=sclt[:, 2:3])

                # ---- softmax denominator (gpsimd) + DMA out ----
                den = xp.tile([P, ST], F32, tag="den", bufs=2)
                dt1 = xp.tile([P, ST], F32, tag="dtmp", bufs=4)
                dt2 = xp.tile([P, ST], F32, tag="dtmp", bufs=4)
                nc.gpsimd.tensor_add(den[:], e_t[:, 0, :], e_t[:, 1, :])
                nc.gpsimd.tensor_add(dt1[:], e_t[:, 2, :], e_t[:, 3, :])
                nc.gpsimd.tensor_add(dt2[:], e_t[:, 4, :], e_t[:, 5, :])
                nc.gpsimd.tensor_add(den[:], den[:], dt1[:])
                nc.gpsimd.tensor_add(den[:], den[:], dt2[:])
                nc.sync.dma_start(d_den[:, s0:s0 + ST], den[0:HD, :])

                # ---- V GEMM + e-weighted task sum -> num, DMA out ----
                for hc in range(NHC):
                    scs = []
                    for p in range(T):
                        c = c_list[p]
                        o = off[p]
                        ps_v = psp.tile([P, ST], F32, tag="ps_v", bufs=2)
                        if use_fp8:
                            for i in range(c // 2):
                                nc.tensor.matmul(
                                    ps_v[:],
                                    wvt[:, o + 2 * i:o + 2 * i + 2,
                                        hc * P:(hc + 1) * P],
                                    gst[:, o + 2 * i:o + 2 * i + 2, :],
                                    start=(i == 0),
                                    stop=(c % 2 == 0 and i == c // 2 - 1),
                                    perf_mode=DR)
                            if c % 2:
                                nc.tensor.matmul(
                                    ps_v[:],
                                    wvt[:, o + c - 1, hc * P:(hc + 1) * P],
                                    gst[:, o + c - 1, :],
                                    start=(c // 2 == 0), stop=True)
                        else:
                            for i in range(c):
                                nc.tensor.matmul(
                                    ps_v[:],
                                    wvt[:, o + i, hc * P:(hc + 1) * P],
                                    gst[:, o + i, :],
                                    start=(i == 0), stop=(i == c - 1))
                        sc = xp.tile([P, ST], BF16, tag="sc", bufs=8)
                        nc.vector.tensor_mul(sc[:], ps_v[:], e_t[:, p, :])
                        scs.append(sc)
                    g1_ = xp.tile([P, ST], BF16, tag="vtmp", bufs=6)
                    g2_ = xp.tile([P, ST], BF16, tag="vtmp", bufs=6)
                    g3_ = xp.tile([P, ST], BF16, tag="vtmp", bufs=6)
                    numt = xp.tile([P, ST], F32, tag="num", bufs=4)
                    nc.vector.tensor_add(g1_[:], scs[0][:], scs[1][:])
                    nc.gpsimd.tensor_add(g2_[:], scs[2][:], scs[3][:])
                    nc.gpsimd.tensor_add(g3_[:], scs[4][:], scs[5][:])
                    nc.vector.tensor_add(g1_[:], g1_[:], g2_[:])
                    nc.vector.tensor_add(numt[:], g1_[:], g3_[:])
                    nc.sync.dma_start(
                        d_num[hc * P:(hc + 1) * P, s0:s0 + ST], numt[:])
    nc.compile()
    return nc


def _sigmoid(x):
    with np.errstate(over="ignore"):
        return 1.0 / (1.0 + np.exp(-x))


def _pow2_scale(arr, target=224.0):
    m = float(np.abs(arr).max())
    if m <= 0.0 or not np.isfinite(m):
        return 1.0
    return float(2.0 ** np.floor(np.log2(target / m)))


def _host_prep(x, fc1_w, fc1_b, fc2_w, fc2_b, efc1, efc2, etask,
               q_w, q_b, k_w, k_b, v_w, v_b, equery, ekey, evalue, t, s):
    f64 = np.float64
    t = int(t)
    s = float(s)
    assert t + 1 == T and x.shape == (B, S, H)
    fc1_w = np.asarray(fc1_w, f64); fc1_b = np.asarray(fc1_b, f64)
    fc2_w = np.asarray(fc2_w, f64); fc2_b = np.asarray(fc2_b, f64)
    efc1 = np.asarray(efc1, f64); efc2 = np.asarray(efc2, f64)
    etask = np.asarray(etask, f64)
    q_w = np.asarray(q_w, f64); q_b = np.asarray(q_b, f64)
    k_w = np.asarray(k_w, f64); k_b = np.asarray(k_b, f64)
    v_w = np.asarray(v_w, f64); v_b = np.asarray(v_b, f64)
    equery = np.asarray(equery, f64); ekey = np.asarray(ekey, f64)
    evalue = np.asarray(evalue, f64)

    g1 = np.stack([_sigmoid(s * efc1[t])] + [_sigmoid(SMAX * efc1[p]) for p in range(t)])
    g2 = np.stack([_sigmoid(s * efc2[t])] + [_sigmoid(SMAX * efc2[p]) for p in range(t)])
    gq = _sigmoid(s * equery[t]); gk = _sigmoid(s * ekey[t]); gv = _sigmoid(s * evalue[t])

    q_vec = (etask[t] @ q_w.T + q_b) * gq
    q_mat = q_vec.reshape(NH, HD)
    kwg = k_w * gk[:, None]
    Mk = np.einsum("nd,ndj->dj", q_mat, kwg.reshape(NH, HD, H)) / np.sqrt(HD)
    ck = np.einsum("nd,nd->d", q_mat, (k_b * gk).reshape(NH, HD)) / np.sqrt(HD)
    MkTdup = np.concatenate([Mk.T, Mk.T], axis=1)            # [H, 128]
    ck_dup = np.tile(ck, 2).astype(np.float32).reshape(P, 1)
    WvT = (v_w * gv[:, None]).T                              # [H, H]
    vbg_perm = (v_b * gv).reshape(NH, HD).T.reshape(H)       # h' = d*16+n
    W2T_raw = fc2_w.T                                        # [A, H]

    # per-task active channels, padded to chunks of 128
    c_list, idx_g, w_g = [], [], []
    for p in range(T):
        idx = np.where(g2[p] > THR)[0]
        c = max(1, int(np.ceil(len(idx) / P)))
        pad = c * P - len(idx)
        c_list.append(c)
        idx_g.append(np.concatenate([idx, np.zeros(pad, np.int64)]))
        w_g.append(np.concatenate([g2[p][idx], np.zeros(pad)]))
    CSUM = sum(c_list)

    zero_bias = bool(np.all(fc1_b == 0.0) and np.all(fc2_b == 0.0))
    use_fp8 = USE_FP8
    if use_fp8:
        qdt = NPFP8

        def q(arr, sc):
            return np.ascontiguousarray(
                np.clip(np.asarray(arr, np.float64) * sc, -240, 240)
            ).astype(qdt)
    else:
        qdt = NPBF16

        def q(arr, sc):
            assert sc == 1.0
            return np.ascontiguousarray(arr).astype(qdt)

    fc1T = fc1_w.T                                           # [H, A]
    W2g = np.empty((A, CSUM * P))
    b2g = np.empty((P, CSUM), np.float32)
    Mkg = np.empty((CSUM * P, P))
    Wvg = np.empty((CSUM * P, H))
    o = 0
    for p in range(T):
        n = c_list[p] * P
        cols = idx_g[p]
        W2g[:, o:o + n] = W2T_raw[:, cols] * g1[p][:, None]
        b2g[:, o // P:(o + n) // P] = np.where(
            w_g[p] > 0, fc2_b[cols], 0.0).reshape(c_list[p], P).T
        Mkg[o:o + n] = MkTdup[cols] * w_g[p][:, None]
        Wvg[o:o + n] = WvT[cols] * w_g[p][:, None]
        o += n

    if use_fp8:
        s_x = _pow2_scale(x)
        s_w1 = _pow2_scale(fc1T)
        s_w2 = _pow2_scale(W2g)
        s_mk = _pow2_scale(Mkg)
        s_v = _pow2_scale(Wvg)
    else:
        s_x = s_w1 = s_w2 = s_mk = s_v = 1.0

    w1h = q(fc1T.reshape(NAC, 2, P, A).transpose(0, 2, 1, 3), s_w1)  # [kp,P,2,A]
    w2h = q(W2g.reshape(NAC, P, CSUM * P).transpose(1, 0, 2), s_w2)  # [P,NAC,CSUM*P]
    mkh = q(Mkg.reshape(CSUM, P, P).transpose(1, 0, 2), s_mk)        # [P,CSUM,P]
    wvh = q(Wvg.reshape(CSUM, P, H).transpose(1, 0, 2), s_v)         # [P,CSUM,H]
    sclh = np.empty((P, 3), np.float32)
    sclh[:, 0] = 1.0 / (s_x * s_w1)
    sclh[:, 1] = 1.0 / s_w2
    sclh[:, 2] = 1.0 / s_mk
    b1h = np.ascontiguousarray(
        fc1_b.reshape(NAC, P).T.astype(np.float32))                  # [P,NAC]

    shared = dict(w1=w1h, b1=b1h, w2=w2h, b2=np.ascontiguousarray(b2g),
                  mk=mkh, wv=wvh, ck=ck_dup, scl=sclh)
    per_core = []
    for b_ in range(B):
        m = dict(shared)
        m["xT"] = q(np.asarray(x[b_], np.float64).T, s_x)
        per_core.append(m)
    post = dict(x=np.asarray(x, np.float32), vbg=vbg_perm.astype(np.float32),
                s_v=s_v, c_list=tuple(c_list), use_fp8=use_fp8,
                zero_bias=zero_bias)
    return per_core, post


def kernel(**inputs):
    in_maps, post = _host_prep(**inputs)
    key = (post["c_list"], post["use_fp8"], post["zero_bias"])
    if _CACHE.get("key") != key:
        _CACHE["nc"] = _build_nc(post["c_list"], post["use_fp8"],
                                 post["zero_bias"])
        _CACHE["key"] = key
    nc = _CACHE["nc"]
    last_err = None
    for _attempt in range(3):
        try:
            res = run_bass_kernel_spmd(nc, in_maps, core_ids=list(range(B)))
            break
        except Exception as e:  # transient NRT device errors: retry
            last_err = e
    else:
        raise last_err
    out = np.empty((B, S, H), np.float32)
    inv_sv = np.float32(1.0 / post["s_v"])
    for b_ in range(B):
        num = res.results[b_]["num"]                  # [H, S] f32, h = n*64+d
        den = res.results[b_]["den"]                  # [HD, S]
        ctx = num.reshape(NH, HD, S) * (inv_sv / den[None, :, :])
        out[b_] = post["x"][b_] + post["vbg"][None, :] \
            + ctx.transpose(2, 1, 0).reshape(S, H)
    return out


# revision 10
# speedup vs baseline: 1.2497x; 1.2497x over previous
"""Trainium2 Bass kernel for nn_BertAdapterAttentionMask (v4: fp8 DR +
gate sparsity + 1-tile software pipelining of the V phase).

Math restructuring:
  * Query is a broadcast task embedding -> scores GEMM is rank-hd with a
    column-duplicated Mk so probs come out partition-duplicated for free.
  * ALL sigmoid gates saturate (s = SMAX = 400), so per-task output gates
    g2 are ~binary. Channels with g2 < 1e-3 are dropped exactly: per task
    the active H-channels are gathered into compact chunks via host-side
    gathers of fc2-output columns / Wv rows / Mk rows (g2 folded into the
    gathered weights). fc2/scores/V shrink from 8 chunks to 4-5 per task.
  * GEMM operands in fp8-e4m3 (TRN FP8_EXP4, max 240) with power-of-2
    per-tensor scales; matmuls in DoubleRow perf mode. Descales ride the
    activation `scale` operand.
  * Device ships the unnormalized attention numerator (sum_t e_t * v_t)
    and denominator (sum_t e_t); softmax division, head permutation,
    V-bias and residual-x add happen on the host.

Scheduling (from v3 trace): the V phase is DVE-bound (48 PSUM-source
muls/s-tile at ~0.7us each), so its emission is deferred one s-tile and
interleaved into the next tile's fc1/fc2 PE stream — PE never waits on
the DVE drain of ps_v. gst/e_t are double-buffered to decouple the
pipelined reads from the next tile's writes. Exps run directly on the
task-paired scores PSUM (2 ACT table switches per s-tile).
"""

import os
import numpy as np
import ml_dtypes
from contextlib import ExitStack

import concourse.bass as bass
import concourse.tile as tile
from concourse import bacc, mybir
from concourse.bass_utils import run_bass_kernel_spmd

AF = mybir.ActivationFunctionType
BF16 = mybir.dt.bfloat16
F32 = mybir.dt.float32
FP8 = mybir.dt.float8e4
NPBF16 = ml_dtypes.bfloat16
NPFP8 = ml_dtypes.float8_e4m3
DR = mybir.MatmulPerfMode.DoubleRow

B, S, H, A, NH, HD = 8, 2048, 1024, 512, 16, 64
T = 6              # tasks = t + 1
P = 128
ST = 512           # s-tile (free-dim tile)
NST = S // ST      # 4
NHC = H // P       # 8
NAC = A // P       # 4
SMAX = 400.0
THR = 1e-3         # drop channels with g2 below this (exact to ~1e-4 abs)

USE_FP8 = os.environ.get("KBENCH_FP8", "1") == "1"

_CACHE = {}


def _build_nc(c_list, use_fp8, zero_bias):
    c_list = list(c_list)
    CSUM = sum(c_list)
    off = [0]
    for c in c_list:
        off.append(off[-1] + c)
    QD = FP8 if use_fp8 else BF16

    nc = bacc.Bacc("TRN2", target_bir_lowering=False, debug=False)

    d_xT = nc.dram_tensor("xT", [H, S], QD, kind="ExternalInput").ap()
    d_w1 = nc.dram_tensor("w1", [NAC, P, 2, A], QD, kind="ExternalInput").ap()
    d_b1 = nc.dram_tensor("b1", [P, NAC], F32, kind="ExternalInput").ap()
    d_w2 = nc.dram_tensor("w2", [P, NAC, CSUM * P], QD, kind="ExternalInput").ap()
    d_b2 = nc.dram_tensor("b2", [P, CSUM], F32, kind="ExternalInput").ap()
    d_mk = nc.dram_tensor("mk", [P, CSUM, P], QD, kind="ExternalInput").ap()
    d_wv = nc.dram_tensor("wv", [P, CSUM, H], QD, kind="ExternalInput").ap()
    d_ck = nc.dram_tensor("ck", [P, 1], F32, kind="ExternalInput").ap()
    d_scl = nc.dram_tensor("scl", [P, 3], F32, kind="ExternalInput").ap()
    d_num = nc.dram_tensor("num", [H, S], BF16, kind="ExternalOutput").ap()
    d_den = nc.dram_tensor("den", [HD, S], F32, kind="ExternalOutput").ap()

    with tile.TileContext(nc) as tc:
        with ExitStack() as ctx:
            wp = ctx.enter_context(tc.tile_pool(name="weights", bufs=1))
            xp = ctx.enter_context(tc.tile_pool(name="acts", bufs=2))
            psp = ctx.enter_context(
                tc.tile_pool(name="psum", bufs=2, space="PSUM")
            )

            # ---- resident weights (DMA order = first-use order) ----
            w1t = []
            xt0 = []
            for kp in range(NAC):
                t_ = wp.tile([P, 2, A], QD, tag=f"w1_{kp}")
                nc.sync.dma_start(t_[:], d_w1[kp])
                w1t.append(t_)
                t_ = xp.tile([P, 2, ST], QD, name=f"xt{kp}", tag=f"xt_{kp}", bufs=2)
                nc.sync.dma_start(t_[:, 0, :], d_xT[kp * 2 * P:kp * 2 * P + P, 0:ST])
                nc.sync.dma_start(t_[:, 1, :], d_xT[kp * 2 * P + P:(kp + 1) * 2 * P, 0:ST])
                xt0.append(t_)
            b1t = wp.tile([P, NAC], F32, tag="b1")
            nc.sync.dma_start(b1t[:], d_b1)
            sclt = wp.tile([P, 3], F32, tag="scl")
            nc.sync.dma_start(sclt[:], d_scl)
            ckt = wp.tile([P, 1], F32, tag="ck")
            nc.sync.dma_start(ckt[:], d_ck)
            w2t = wp.tile([P, NAC, CSUM * P], QD, tag="w2")
            for p in range(T):
                nc.sync.dma_start(
                    w2t[:, :, off[p] * P:off[p + 1] * P],
                    d_w2[:, :, off[p] * P:off[p + 1] * P],
                )
            b2t = wp.tile([P, CSUM], F32, tag="b2")
            nc.sync.dma_start(b2t[:], d_b2)
            mkt = wp.tile([P, CSUM, P], QD, tag="mk")
            nc.sync.dma_start(mkt[:], d_mk)
            wvt = wp.tile([P, CSUM, H], QD, tag="wv")
            for p in range(T):
                nc.sync.dma_start(
                    wvt[:, off[p]:off[p + 1], :], d_wv[:, off[p]:off[p + 1], :]
                )

            scs_by_hc = {}   # hc -> list of sc tiles (filled as closures run)

            def emit_v_one(gst_, e_, s0_, hc, p):
                """V GEMM + e-weighting for one (h-chunk, task); the last
                task also emits the task-sum tree + num DMA (deferred)."""
                c = c_list[p]
                o = off[p]
                ps_v = psp.tile([P, ST], F32, tag="ps_v", bufs=2)
                if use_fp8:
                    for i in range(c // 2):
                        nc.tensor.matmul(
                            ps_v[:],
                            wvt[:, o + 2 * i:o + 2 * i + 2,
                                hc * P:(hc + 1) * P],
                            gst_[:, o + 2 * i:o + 2 * i + 2, :],
                            start=(i == 0),
                            stop=(c % 2 == 0 and i == c // 2 - 1),
                            perf_mode=DR)
                    if c % 2:
                        nc.tensor.matmul(
                            ps_v[:],
                            wvt[:, o + c - 1, hc * P:(hc + 1) * P],
                            gst_[:, o + c - 1, :],
                            start=(c // 2 == 0), stop=True)
                else:
                    for i in range(c):
                        nc.tensor.matmul(
                            ps_v[:],
                            wvt[:, o + i, hc * P:(hc + 1) * P],
                            gst_[:, o + i, :],
                            start=(i == 0), stop=(i == c - 1))
                sc = xp.tile([P, ST], BF16, tag="sc", bufs=14)
                nc.vector.tensor_mul(sc[:], ps_v[:], e_[:, p, :])
                scs_by_hc.setdefault(hc, []).append(sc)
                if p == T - 1:
                    scs = scs_by_hc.pop(hc)
                    g1_ = xp.tile([P, ST], BF16, tag="vtmp", bufs=6)
                    g2_ = xp.tile([P, ST], BF16, tag="vtmp", bufs=6)
                    g3_ = xp.tile([P, ST], BF16, tag="vtmp", bufs=6)
                    numt = xp.tile([P, ST], BF16, tag="num", bufs=4)
                    nc.vector.tensor_add(g1_[:], scs[0][:], scs[1][:])
                    nc.gpsimd.tensor_add(g2_[:], scs[2][:], scs[3][:])
                    nc.gpsimd.tensor_add(g3_[:], scs[4][:], scs[5][:])
                    nc.gpsimd.tensor_add(g1_[:], g1_[:], g2_[:])
                    nc.vector.tensor_add(numt[:], g1_[:], g3_[:])
                    nc.sync.dma_start(
                        d_num[hc * P:(hc + 1) * P, s0_:s0_ + ST], numt[:])

            pending_v = []   # deferred V-phase emitters from the previous tile

            def flush_v(k):
                for _ in range(k):
                    if pending_v:
                        pending_v.pop(0)()

            for st in range(NST):
                s0 = st * ST
                if st == 0:
                    xt = xt0
                else:
                    xt = []
                    for kp in range(NAC):
                        t_ = xp.tile([P, 2, ST], QD, name=f"xt{kp}",
                                     tag=f"xt_{kp}", bufs=2)
                        nc.sync.dma_start(
                            t_[:, 0, :],
                            d_xT[kp * 2 * P:kp * 2 * P + P, s0:s0 + ST])
                        nc.sync.dma_start(
                            t_[:, 1, :],
                            d_xT[kp * 2 * P + P:(kp + 1) * 2 * P, s0:s0 + ST])
                        xt.append(t_)

                # ---- fc1 -> h1 (gelu), 2-wide PSUM ----
                h1 = xp.tile([P, NAC, ST], QD, name="h1", tag="h1", bufs=2)
                for a2 in range(NAC // 2):
                    ps2 = psp.tile([P, 2, ST], F32, tag="ps_mm", bufs=2)
                    for half in range(2):
                        ac = a2 * 2 + half
                        for kp in range(NAC):
                            if use_fp8:
                                nc.tensor.matmul(
                                    ps2[:, half, :],
                                    w1t[kp][:, :, ac * P:(ac + 1) * P],
                                    xt[kp][:],
                                    start=(kp == 0), stop=(kp == NAC - 1),
                                    perf_mode=DR)
                            else:
                                for i in range(2):
                                    nc.tensor.matmul(
                                        ps2[:, half, :],
                                        w1t[kp][:, i, ac * P:(ac + 1) * P],
                                        xt[kp][:, i, :],
                                        start=(kp == 0 and i == 0),
                                        stop=(kp == NAC - 1 and i == 1))
                    if zero_bias:
                        nc.scalar.activation(
                            h1[:, a2 * 2:a2 * 2 + 2, :], ps2[:], AF.Gelu,
                            scale=sclt[:, 0:1])
                    else:
                        for half in range(2):
                            ac = a2 * 2 + half
                            nc.scalar.activation(
                                h1[:, ac, :], ps2[:, half, :], AF.Gelu,
                                bias=b1t[:, ac:ac + 1], scale=sclt[:, 0:1])
                    flush_v(4)

                # ---- fc2 per task (2-wide PSUM + paired gelus) ----
                gst = xp.tile([P, CSUM, ST], QD, name="gst", tag="gst", bufs=2)
                for p in range(T):
                    c = c_list[p]
                    o = off[p]
                    for c2 in range(0, c, 2):
                        wide = 2 if c2 + 1 < c else 1
                        ps2 = psp.tile([P, 2, ST], F32, tag="ps_mm", bufs=2)
                        for half in range(wide):
                            j = o + c2 + half
                            if use_fp8:
                                nc.tensor.matmul(
                                    ps2[:, half, :],
                                    w2t[:, 0:2, j * P:(j + 1) * P],
                                    h1[:, 0:2, :], start=True, stop=False,
                                    perf_mode=DR)
                                nc.tensor.matmul(
                                    ps2[:, half, :],
                                    w2t[:, 2:4, j * P:(j + 1) * P],
                                    h1[:, 2:4, :], start=False, stop=True,
                                    perf_mode=DR)
                            else:
                                for a4 in range(NAC):
                                    nc.tensor.matmul(
                                        ps2[:, half, :],
                                        w2t[:, a4, j * P:(j + 1) * P],
                                        h1[:, a4, :],
                                        start=(a4 == 0),
                                        stop=(a4 == NAC - 1))
                        if zero_bias and wide == 2:
                            nc.scalar.activation(
                                gst[:, o + c2:o + c2 + 2, :], ps2[:],
                                AF.Gelu, scale=sclt[:, 1:2])
                        else:
                            for half in range(wide):
                                j = o + c2 + half
                                nc.scalar.activation(
                                    gst[:, j, :], ps2[:, half, :], AF.Gelu,
                                    bias=b2t[:, j:j + 1], scale=sclt[:, 1:2])
                        flush_v(2)

                # ---- scores -> exp (exps contiguous: one table switch) ----
                e_t = xp.tile([P, T, ST], F32, name="e", tag="e", bufs=2)
                for p in range(T):
                    c = c_list[p]
                    o = off[p]
                    ps_s = psp.tile([P, ST], F32, tag="ps_s", bufs=2)
                    if use_fp8:
                        for i in range(c // 2):
                            nc.tensor.matmul(
                                ps_s[:],
                                mkt[:, o + 2 * i:o + 2 * i + 2, :],
                                gst[:, o + 2 * i:o + 2 * i + 2, :],
                                start=(i == 0),
                                stop=(c % 2 == 0 and i == c // 2 - 1),
                                perf_mode=DR)
                        if c % 2:
                            nc.tensor.matmul(
                                ps_s[:], mkt[:, o + c - 1, :],
                                gst[:, o + c - 1, :],
                                start=(c // 2 == 0), stop=True)
                    else:
                        for i in range(c):
                            nc.tensor.matmul(
                                ps_s[:], mkt[:, o + i, :], gst[:, o + i, :],
                                start=(i == 0), stop=(i == c - 1))
                    nc.scalar.activation(
                        e_t[:, p, :], ps_s[:],
                        AF.Exp, bias=ckt[:], scale=sclt[:, 2:3])
                    flush_v(2)

                # ---- softmax denominator (gpsimd) + DMA out ----
                den = xp.tile([P, ST], F32, tag="den", bufs=2)
                dt1 = xp.tile([P, ST], F32, tag="dtmp", bufs=4)
                dt2 = xp.tile([P, ST], F32, tag="dtmp", bufs=4)
                nc.gpsimd.tensor_add(den[:], e_t[:, 0, :], e_t[:, 1, :])
                nc.gpsimd.tensor_add(dt1[:], e_t[:, 2, :], e_t[:, 3, :])
                nc.gpsimd.tensor_add(dt2[:], e_t[:, 4, :], e_t[:, 5, :])
                nc.gpsimd.tensor_add(den[:], den[:], dt1[:])
                nc.gpsimd.tensor_add(den[:], den[:], dt2[:])
                nc.sync.dma_start(d_den[:, s0:s0 + ST], den[0:HD, :])

                # remaining V of the previous tile, then queue this tile's V
                flush_v(len(pending_v))
                for hc in range(NHC):
                    for p in range(T):
                        pending_v.append(
                            lambda g_=gst, e_=e_t, s_=s0, h_=hc, p_=p:
                                emit_v_one(g_, e_, s_, h_, p_))
            flush_v(len(pending_v))
    nc.compile()
    return nc


def _sigmoid(x):
    with np.errstate(over="ignore"):
        return 1.0 / (1.0 + np.exp(-x))


def _pow2_scale(arr, target=224.0):
    m = float(np.abs(arr).max())
    if m <= 0.0 or not np.isfinite(m):
        return 1.0
    return float(2.0 ** np.floor(np.log2(target / m)))


def _host_prep(x, fc1_w, fc1_b, fc2_w, fc2_b, efc1, efc2, etask,
               q_w, q_b, k_w, k_b, v_w, v_b, equery, ekey, evalue, t, s):
    f64 = np.float64
    t = int(t)
    s = float(s)
    assert t + 1 == T and x.shape == (B, S, H)
    fc1_w = np.asarray(fc1_w, f64); fc1_b = np.asarray(fc1_b, f64)
    fc2_w = np.asarray(fc2_w, f64); fc2_b = np.asarray(fc2_b, f64)
    efc1 = np.asarray(efc1, f64); efc2 = np.asarray(efc2, f64)
    etask = np.asarray(etask, f64)
    q_w = np.asarray(q_w, f64); q_b = np.asarray(q_b, f64)
    k_w = np.asarray(k_w, f64); k_b = np.asarray(k_b, f64)
    v_w = np.asarray(v_w, f64); v_b = np.asarray(v_b, f64)
    equery = np.asarray(equery, f64); ekey = np.asarray(ekey, f64)
    evalue = np.asarray(evalue, f64)

    g1 = np.stack([_sigmoid(s * efc1[t])] + [_sigmoid(SMAX * efc1[p]) for p in range(t)])
    g2 = np.stack([_sigmoid(s * efc2[t])] + [_sigmoid(SMAX * efc2[p]) for p in range(t)])
    gq = _sigmoid(s * equery[t]); gk = _sigmoid(s * ekey[t]); gv = _sigmoid(s * evalue[t])

    q_vec = (etask[t] @ q_w.T + q_b) * gq
    q_mat = q_vec.reshape(NH, HD)
    kwg = k_w * gk[:, None]
    Mk = np.einsum("nd,ndj->dj", q_mat, kwg.reshape(NH, HD, H)) / np.sqrt(HD)
    ck = np.einsum("nd,nd->d", q_mat, (k_b * gk).reshape(NH, HD)) / np.sqrt(HD)
    MkTdup = np.concatenate([Mk.T, Mk.T], axis=1)            # [H, 128]
    ck_dup = np.tile(ck, 2).astype(np.float32).reshape(P, 1)
    WvT = (v_w * gv[:, None]).T                              # [H, H]
    vbg_perm = (v_b * gv).reshape(NH, HD).T.reshape(H)       # h' = d*16+n
    W2T_raw = fc2_w.T                                        # [A, H]

    # per-task active channels, padded to chunks of 128
    c_list, idx_g, w_g = [], [], []
    for p in range(T):
        idx = np.where(g2[p] > THR)[0]
        c = max(1, int(np.ceil(len(idx) / P)))
        pad = c * P - len(idx)
        c_list.append(c)
        idx_g.append(np.concatenate([idx, np.zeros(pad, np.int64)]))
        w_g.append(np.concatenate([g2[p][idx], np.zeros(pad)]))
    CSUM = sum(c_list)

    zero_bias = bool(np.all(fc1_b == 0.0) and np.all(fc2_b == 0.0))
    use_fp8 = USE_FP8
    if use_fp8:
        qdt = NPFP8

        def q(arr, sc):
            return np.ascontiguousarray(
                np.clip(np.asarray(arr, np.float64) * sc, -240, 240)
            ).astype(qdt)
    else:
        qdt = NPBF16

        def q(arr, sc):
            assert sc == 1.0
            return np.ascontiguousarray(arr).astype(qdt)

    fc1T = fc1_w.T                                           # [H, A]
    W2g = np.empty((A, CSUM * P))
    b2g = np.empty((P, CSUM), np.float32)
    Mkg = np.empty((CSUM * P, P))
    Wvg = np.empty((CSUM * P, H))
    o = 0
    for p in range(T):
        n = c_list[p] * P
        cols = idx_g[p]
        W2g[:, o:o + n] = W2T_raw[:, cols] * g1[p][:, None]
        b2g[:, o // P:(o + n) // P] = np.where(
            w_g[p] > 0, fc2_b[cols], 0.0).reshape(c_list[p], P).T
        Mkg[o:o + n] = MkTdup[cols] * w_g[p][:, None]
        Wvg[o:o + n] = WvT[cols] * w_g[p][:, None]
        o += n

    if use_fp8:
        s_x = _pow2_scale(x)
        s_w1 = _pow2_scale(fc1T)
        s_w2 = _pow2_scale(W2g)
        s_mk = _pow2_scale(Mkg)
        s_v = _pow2_scale(Wvg)
    else:
        s_x = s_w1 = s_w2 = s_mk = s_v = 1.0

    w1h = q(fc1T.reshape(NAC, 2, P, A).transpose(0, 2, 1, 3), s_w1)  # [kp,P,2,A]
    w2h = q(W2g.reshape(NAC, P, CSUM * P).transpose(1, 0, 2), s_w2)  # [P,NAC,CSUM*P]
    mkh = q(Mkg.reshape(CSUM, P, P).transpose(1, 0, 2), s_mk)        # [P,CSUM,P]
    wvh = q(Wvg.reshape(CSUM, P, H).transpose(1, 0, 2), s_v)         # [P,CSUM,H]
    sclh = np.empty((P, 3), np.float32)
    sclh[:, 0] = 1.0 / (s_x * s_w1)
    sclh[:, 1] = 1.0 / s_w2
    sclh[:, 2] = 1.0 / s_mk
    b1h = np.ascontiguousarray(
        fc1_b.reshape(NAC, P).T.astype(np.float32))                  # [P,NAC]

    shared = dict(w1=w1h, b1=b1h, w2=w2h, b2=np.ascontiguousarray(b2g),
                  mk=mkh, wv=wvh, ck=ck_dup, scl=sclh)
    per_core = []
    for b_ in range(B):
        m = dict(shared)
        m["xT"] = q(np.asarray(x[b_], np.float64).T, s_x)
        per_core.append(m)
    post = dict(x=np.asarray(x, np.float32), vbg=vbg_perm.astype(np.float32),
                s_v=s_v, c_list=tuple(c_list), use_fp8=use_fp8,
                zero_bias=zero_bias)
    return per_core, post


def kernel(**inputs):
    in_maps, post = _host_prep(**inputs)
    key = (post["c_list"], post["use_fp8"], post["zero_bias"])
    if _CACHE.get("key") != key:
        _CACHE["nc"] = _build_nc(post["c_list"], post["use_fp8"],
                                 post["zero_bias"])
        _CACHE["key"] = key
    nc = _CACHE["nc"]
    last_err = None
    for _attempt in range(3):
        try:
            res = run_bass_kernel_spmd(nc, in_maps, core_ids=list(range(B)))
            break
        except Exception as e:  # transient NRT device errors: retry
            last_err = e
    else:
        raise last_err
    out = np.empty((B, S, H), np.float32)
    inv_sv = np.float32(1.0 / post["s_v"])
    for b_ in range(B):
        num = np.asarray(res.results[b_]["num"], np.float32)  # [H,S], h=n*64+d
        den = np.asarray(res.results[b_]["den"], np.float32)  # [HD, S]
        ctx = num.reshape(NH, HD, S) * (inv_sv / den[None, :, :])
        out[b_] = post["x"][b_] + post["vbg"][None, :] \
            + ctx.transpose(2, 1, 0).reshape(S, H)
    return out
